# revision 50
# baseline (speedup 1.0000x reference)
"""Trainium2 Bass/Tile kernel for the contrastive (NT-Xent-style) loss.

reference math (B=8192, D=1024, K=30):
    zi = l2norm(z_i); zj = l2norm(z_j)
    pos = rowdot(zi, zj)                      # [B]
    pool = cat(zi, zj)                        # [2B, D]
    neg[b,k] = dot(zi[b], pool[idx[b,k]])     # [B,K]
    logits = cat(pos, neg)/T; loss = mean_b(logsumexp(logits_b) - logits_b0)

Distribution: data-parallel over B across 8 cores (1024 rows each).

Launch 1 (per core): reads its z_i/z_j row shard (f32), computes row norms
(ACT square+fused-rowsum, ACT sqrt, DVE reciprocal), writes bf16 normalized
shards + f32 pos-dots [128, 8] (partition=row%128, col=row//128). The host
concatenates the 16 shard outputs into the full normalized pool (pure data
movement, no math).

Launch 2 (per core): the full normalized bf16 pool is replicated; for each
128-row subtile one dma_gather(transpose=True) pulls its 3840 negative rows
in d-transposed layout [128=d%128, 8=d//128, 3840]. TensorE computes
block-diagonal query/negative dots: 4 groups of 32 queries vs their 960
gathered columns, output col-tiled to PSUM partitions 32j..32j+31, contracted
over 8 d-chunks, split in two 480-wide halves (PSUM bank limit). The PSUM is
multiplied by a 0/1 mask (zeroing the off-diagonal waste entries), then
ScalarE computes exp(x/T) with a fused row-sum. Off-diagonal zeros contribute
exp(0)=1 each, so row_loss = log(exp(p/T) + sum - 930) - p/T. Row losses
accumulate across subtiles and reduce to one scalar per core via a
ones-vector matmul; the host sums 8 scalars and divides by B.
"""

import sys

for _p in ("/opt/trn_rl_repo",):
    if _p not in sys.path:
        sys.path.insert(0, _p)

import numpy as np
import ml_dtypes  # noqa: F401  (bf16 numpy dtype registration)

import concourse.bacc as bacc
import concourse.bass as bass  # noqa: F401
import concourse.mybir as mybir
import concourse.tile as tile
from concourse.bass_utils import run_bass_kernel_spmd
from concourse.library_config import mlp


def _new_nc():
    return bacc.Bacc("TRN2", target_bir_lowering=False, debug=False)

F32 = mybir.dt.float32
BF16 = mybir.dt.bfloat16
I16 = mybir.dt.int16
AF = mybir.ActivationFunctionType
ALU = mybir.AluOpType

B, D, K = 8192, 1024, 30
M = 8  # cores
BS = B // M  # 1024 rows per core
NSUB = BS // 128  # 8 subtiles of 128 rows
NC_D = D // 128  # 8 d-chunks
KP = 32  # K padded to 32 so gather chunks (128 idx) align with matmul slices
G = 32  # queries per matmul group (one col-tile)
NG = 128 // G  # 4 groups per subtile
GC = G * KP  # 1024 gathered columns per group
NH = 2  # halves per group (PSUM bank limit: 512 f32)
HC = GC // NH  # 512
NIDX = 128 * KP  # 4096 gathered rows per subtile
NCH = NIDX // 128  # 32 gather chunks (128 idx each) per subtile
NCHH = NCH // 2  # 16 chunks per half-subtile gather tile
MASKED_ONES = float(GC - K)  # 994 exp(0)=1 entries per row


def _idx_wrap(vals16):
    """Pack a linear int16 index list into the dma_gather SBUF layout:
    [128, n/16] where linear j sits at (partition j%16, col j//16),
    replicated across the 8 Q7 groups of 16 partitions."""
    n = vals16.shape[0]
    assert n % 16 == 0
    base = vals16.reshape(n // 16, 16).T.astype(np.int16)  # [16, n/16]
    return np.ascontiguousarray(np.tile(base, (8, 1)))  # [128, n/16]


# --------------------------------------------------------------------------
# Launch 1: per-core normalization of the pool shard + pos dots
# --------------------------------------------------------------------------
def build_l1():
    nc = _new_nc()
    zi = nc.dram_tensor("zi", [BS, D], F32, kind="ExternalInput")
    zj = nc.dram_tensor("zj", [BS, D], F32, kind="ExternalInput")
    zin = nc.dram_tensor("zin", [BS, D], BF16, kind="ExternalOutput")
    zjn = nc.dram_tensor("zjn", [BS, D], BF16, kind="ExternalOutput")
    pos8 = nc.dram_tensor("pos8", [128, NSUB], F32, kind="ExternalOutput")

    with tile.TileContext(nc) as tc:
        with (
            tc.tile_pool(name="big", bufs=3) as bigp,
            tc.tile_pool(name="stat", bufs=3) as statp,
            tc.tile_pool(name="posp", bufs=1) as posp,
        ):
            pos_sb = posp.tile([128, NSUB], F32)
            for t in range(NSUB):
                r0 = t * 128
                bi = bigp.tile([128, D], F32, tag="bi")
                nc.sync.dma_start(bi[:, :], zi[r0 : r0 + 128, :])
                bj = bigp.tile([128, D], F32, tag="bj")
                nc.sync.dma_start(bj[:, :], zj[r0 : r0 + 128, :])

                sqdi = bigp.tile([128, D], F32, tag="sqdi")
                ssi = statp.tile([128, 1], F32, tag="ssi")
                nc.scalar.activation(
                    sqdi[:, :], bi[:, :], AF.Square, accum_out=ssi[:, :]
                )
                sqdj = bigp.tile([128, D], F32, tag="sqdj")
                ssj = statp.tile([128, 1], F32, tag="ssj")
                nc.scalar.activation(
                    sqdj[:, :], bj[:, :], AF.Square, accum_out=ssj[:, :]
                )
                nrmi = statp.tile([128, 1], F32, tag="nrmi")
                nc.scalar.activation(nrmi[:, :], ssi[:, :], AF.Sqrt)
                nrmj = statp.tile([128, 1], F32, tag="nrmj")
                nc.scalar.activation(nrmj[:, :], ssj[:, :], AF.Sqrt)
                invi = statp.tile([128, 1], F32, tag="invi")
                nc.vector.reciprocal(invi[:, :], nrmi[:, :])
                invj = statp.tile([128, 1], F32, tag="invj")
                nc.vector.reciprocal(invj[:, :], nrmj[:, :])

                prodd = bigp.tile([128, D], F32, tag="prodd")
                rawdot = statp.tile([128, 1], F32, tag="rawdot")
                nc.vector.tensor_tensor(prodd[:, :], bi[:, :], bj[:, :], ALU.mult)
                nc.vector.tensor_reduce(
                    rawdot[:, :], prodd[:, :], mybir.AxisListType.X, ALU.add
                )

                oi = bigp.tile([128, D], BF16, tag="oi")
                nc.vector.tensor_scalar_mul(oi[:, :], bi[:, :], invi[:, :])
                nc.sync.dma_start(zin[r0 : r0 + 128, :], oi[:, :])
                oj = bigp.tile([128, D], BF16, tag="oj")
                nc.vector.tensor_scalar_mul(oj[:, :], bj[:, :], invj[:, :])
                nc.sync.dma_start(zjn[r0 : r0 + 128, :], oj[:, :])

                tmp = statp.tile([128, 1], F32, tag="tmp")
                nc.vector.tensor_tensor(
                    tmp[:, :], rawdot[:, :], invi[:, :], ALU.mult
                )
                nc.vector.tensor_tensor(
                    pos_sb[:, t : t + 1], tmp[:, :], invj[:, :], ALU.mult
                )
            nc.sync.dma_start(pos8[:, :], pos_sb[:, :])
    nc.compile()
    return nc


# --------------------------------------------------------------------------
# Launch 2: gather + block-diagonal dots + masked softmax-CE
# --------------------------------------------------------------------------
def build_l2():
    nc = _new_nc()
    pool = nc.dram_tensor("pool", [2 * B, D], BF16, kind="ExternalInput")
    ziq_h = nc.dram_tensor("ziq", [BS, D], BF16, kind="ExternalInput")
    idxg_h = nc.dram_tensor("idxg", [128, NSUB * NIDX // 16], I16, kind="ExternalInput")
    pos8_h = nc.dram_tensor("pos8", [128, NSUB], F32, kind="ExternalInput")
    mask_h = nc.dram_tensor("mask", [128, GC], F32, kind="ExternalInput")
    trep_h = nc.dram_tensor("trep", [128, 1], F32, kind="ExternalInput")
    out_h = nc.dram_tensor("out", [1, 1], F32, kind="ExternalOutput")

    with tile.TileContext(nc) as tc:
        with (
            tc.tile_pool(name="const", bufs=1) as cp,
            tc.tile_pool(name="gtp", bufs=4) as gtp,
            tc.tile_pool(name="yp", bufs=2) as yp,
            tc.tile_pool(name="dp", bufs=2) as dp,
            tc.tile_pool(name="rowp", bufs=2) as rowp,
            tc.tile_pool(name="psp", bufs=2, space="PSUM") as psp,
            tc.tile_pool(name="psf", bufs=1, space="PSUM") as psfp,
        ):
            idxg = cp.tile([128, NSUB * NIDX // 16], I16)
            nc.sync.dma_start(idxg[:, :], idxg_h[:, :])
            pos8 = cp.tile([128, NSUB], F32)
            nc.sync.dma_start(pos8[:, :], pos8_h[:, :])
            mask = cp.tile([128, GC], F32)
            nc.sync.dma_start(mask[:, :], mask_h[:, :])
            trep = cp.tile([128, 1], F32)
            nc.sync.dma_start(trep[:, :], trep_h[:, :])

            invt = cp.tile([128, 1], F32)
            nc.vector.reciprocal(invt[:, :], trep[:, :])
            ones = cp.tile([128, 1], F32)
            nc.vector.memset(ones[:, :], 1.0)
            s8 = cp.tile([128, NSUB], F32)
            p8 = cp.tile([128, NSUB], F32)

            nc.gpsimd.load_library(mlp)
            nidx_reg = nc.gpsimd.alloc_register("nidx128")
            nc.gpsimd.reg_mov(nidx_reg, 128)
            # queries^T via HWDGE xbar transpose (affine; keeps gpsimd free):
            # ziT[p, t, c, q] = zi_n[t*128+q, c*128+p]
            ziT = cp.tile([128, NSUB, NC_D, 128], BF16)
            for t in range(NSUB):
                for c in range(NC_D):
                    nc.sync.dma_start(
                        ziT[:, t, c, :],
                        ziq_h[t * 128 : (t + 1) * 128, c * 128 : (c + 1) * 128],
                        transpose=True,
                    )

            for t in range(NSUB):
                # two half-subtile gather tiles of 16 chunks x 128 rows each
                gts = []
                for s in range(2):
                    gth = gtp.tile([128, NCHH, NC_D, 128], BF16, tag="gt")
                    for ci in range(NCHH):
                        col0 = (t * NCH + s * NCHH + ci) * 8
                        nc.gpsimd.dma_gather(
                            gth[:, ci],
                            pool[:, :],
                            idxg[:, col0 : col0 + 8],
                            128,
                            nidx_reg,
                            D,
                            transpose=True,
                        )
                    gts.append(gth)
                ps_a = psp.tile([128, HC], F32, tag="psa")
                ps_b = psp.tile([128, HC], F32, tag="psb")
                pss = [ps_a, ps_b]
                for j in range(NG):
                    gth = gts[j // 2]
                    jl = j % 2  # group index within the half tile
                    for h in range(NH):
                        cs = jl * (NCHH // 2) + h * (NCHH // 4)  # 8*jl + 4*h
                        for c in range(NC_D):
                            nc.tensor.matmul(
                                pss[h][j * G : (j + 1) * G, :],
                                ziT[:, t, c, j * G : (j + 1) * G],
                                gth[:, cs : cs + 4, c, :],
                                start=(c == 0),
                                stop=(c == NC_D - 1),
                                tile_position=(0, j * G),
                            )
                # masked exp-sum over both halves
                ss = []
                for h in range(NH):
                    y = yp.tile([128, HC], F32, tag=f"y{h}")
                    nc.vector.tensor_tensor(
                        y[:, :],
                        pss[h][:, :],
                        mask[:, h * HC : (h + 1) * HC],
                        ALU.mult,
                    )
                    ed = dp.tile([128, HC], F32, tag=f"ed{h}")
                    s = rowp.tile([128, 1], F32, tag=f"s{h}")
                    nc.scalar.activation(
                        ed[:, :],
                        y[:, :],
                        AF.Exp,
                        scale=invt[:, :],
                        accum_out=s[:, :],
                    )
                    ss.append(s)
                nc.vector.tensor_scalar_mul(
                    p8[:, t : t + 1], pos8[:, t : t + 1], invt[:, :]
                )
                nc.vector.tensor_tensor(
                    s8[:, t : t + 1], ss[0][:, :], ss[1][:, :], ALU.add
                )

            # row losses, batched: loss = ln(s - 994 + exp(p)) - p
            ep8 = cp.tile([128, NSUB], F32)
            nc.scalar.activation(ep8[:, :], p8[:, :], AF.Exp)
            tot8 = cp.tile([128, NSUB], F32)
            nc.vector.tensor_tensor(tot8[:, :], s8[:, :], ep8[:, :], ALU.add)
            tot28 = cp.tile([128, NSUB], F32)
            nc.vector.tensor_scalar_add(tot28[:, :], tot8[:, :], -MASKED_ONES)
            lg8 = cp.tile([128, NSUB], F32)
            nc.scalar.activation(lg8[:, :], tot28[:, :], AF.Ln)
            d8 = cp.tile([128, NSUB], F32)
            nc.vector.tensor_tensor(d8[:, :], lg8[:, :], p8[:, :], ALU.subtract)
            rowsum = cp.tile([128, 1], F32)
            nc.vector.tensor_reduce(
                rowsum[:, :], d8[:, :], mybir.AxisListType.X, ALU.add
            )

            psf = psfp.tile([128, 16], F32)
            nc.tensor.matmul(
                psf[0:1, 0:1], ones[:, :], rowsum[:, :], start=True, stop=True
            )
            res = cp.tile([1, 1], F32)
            nc.vector.tensor_copy(res[:, :], psf[0:1, 0:1])
            nc.sync.dma_start(out_h[:, :], res[:, :])
    nc.compile()
    return nc


# --------------------------------------------------------------------------
# Host-side driver
# --------------------------------------------------------------------------
def _build_l2_inputs(pool_n, pos8s, neg_indices, temperature):
    """Per-core input dicts for launch 2 (pure data movement)."""
    mask = np.zeros((128, GC), dtype=np.float32)
    p = np.arange(128)[:, None]
    col = np.arange(GC)[None, :]
    mask[((col // KP) == (p % G)) & ((col % KP) < K)] = 1.0
    trep = np.full((128, 1), temperature, dtype=np.float32)

    # subtile pair order: P = b_local*KP + kk, kk >= K padded with kk=0's idx
    X = np.arange(NIDX)
    bl = X // KP  # b_local within subtile
    kk = np.minimum(X % KP, K - 1) * ((X % KP) < K)  # pad slots reuse kk=0

    in_maps = []
    for m in range(M):
        idxg = np.empty((NSUB * NIDX,), dtype=np.int64)
        for t in range(NSUB):
            rows = m * BS + t * 128 + bl
            idxg[t * NIDX : (t + 1) * NIDX] = neg_indices[rows, kk]
        idxg_w = _idx_wrap(idxg.astype(np.int16))
        in_maps.append(
            {
                "pool": pool_n,
                "ziq": np.ascontiguousarray(pool_n[m * BS : (m + 1) * BS]),
                "idxg": idxg_w,
                "pos8": pos8s[m],
                "mask": mask,
                "trep": trep,
            }
        )
    return in_maps


def kernel(z_i, z_j, temperature, neg_indices, _timing=None):
    z_i = np.ascontiguousarray(np.asarray(z_i, dtype=np.float32))
    z_j = np.ascontiguousarray(np.asarray(z_j, dtype=np.float32))
    neg_indices = np.asarray(neg_indices)
    temp = float(np.asarray(temperature))

    cores = list(range(M))
    trace = _timing is not None

    # ---- launch 1
    nc1 = build_l1()
    in1 = [
        {
            "zi": z_i[m * BS : (m + 1) * BS],
            "zj": z_j[m * BS : (m + 1) * BS],
        }
        for m in range(M)
    ]
    r1 = run_bass_kernel_spmd(nc1, in1, cores, trace=trace)
    if trace:
        _timing.append(("l1", r1.exec_time_ns))

    pool_n = np.ascontiguousarray(
        np.concatenate(
            [r1.results[m]["zin"] for m in range(M)]
            + [r1.results[m]["zjn"] for m in range(M)],
            axis=0,
        )
    )
    pos8s = [np.ascontiguousarray(r1.results[m]["pos8"]) for m in range(M)]

    # ---- launch 2
    nc2 = build_l2()
    in2 = _build_l2_inputs(pool_n, pos8s, neg_indices, temp)
    r2 = run_bass_kernel_spmd(nc2, in2, cores, trace=trace)
    if trace:
        _timing.append(("l2", r2.exec_time_ns))

    total = np.float64(0.0)
    for m in range(M):
        total += np.float64(r2.results[m]["out"][0, 0])
    return np.float32(total / B)


# revision 54
# speedup vs baseline: 1.3758x; 1.3758x over previous
"""Trainium2 Bass/Tile kernel for the contrastive (NT-Xent-style) loss.

reference math (B=8192, D=1024, K=30):
    zi = l2norm(z_i); zj = l2norm(z_j)
    pos = rowdot(zi, zj)                      # [B]
    pool = cat(zi, zj)                        # [2B, D]
    neg[b,k] = dot(zi[b], pool[idx[b,k]])     # [B,K]
    logits = cat(pos, neg)/T; loss = mean_b(logsumexp(logits_b) - logits_b0)

Distribution: data-parallel over B across 8 cores (1024 rows each).

Launch 1 (per core): reads its z_i/z_j row shard (f32), computes row norms
(ACT square+fused-rowsum, ACT sqrt, DVE reciprocal), writes bf16 normalized
shards + f32 pos-dots [128, 8] (partition=row%128, col=row//128). The host
concatenates the 16 shard outputs into the full normalized pool (pure data
movement, no math).

Launch 2 (per core): the full normalized bf16 pool is replicated; for each
128-row subtile one dma_gather(transpose=True) pulls its 3840 negative rows
in d-transposed layout [128=d%128, 8=d//128, 3840]. TensorE computes
block-diagonal query/negative dots: 4 groups of 32 queries vs their 960
gathered columns, output col-tiled to PSUM partitions 32j..32j+31, contracted
over 8 d-chunks, split in two 480-wide halves (PSUM bank limit). The PSUM is
multiplied by a 0/1 mask (zeroing the off-diagonal waste entries), then
ScalarE computes exp(x/T) with a fused row-sum. Off-diagonal zeros contribute
exp(0)=1 each, so row_loss = log(exp(p/T) + sum - 930) - p/T. Row losses
accumulate across subtiles and reduce to one scalar per core via a
ones-vector matmul; the host sums 8 scalars and divides by B.
"""

import sys

for _p in ("/opt/trn_rl_repo",):
    if _p not in sys.path:
        sys.path.insert(0, _p)

import numpy as np
import ml_dtypes  # noqa: F401  (bf16 numpy dtype registration)

import concourse.bacc as bacc
import concourse.bass as bass  # noqa: F401
import concourse.mybir as mybir
import concourse.tile as tile
from concourse.bass_utils import run_bass_kernel_spmd
from concourse.library_config import mlp


def _new_nc():
    return bacc.Bacc("TRN2", target_bir_lowering=False, debug=False)

F32 = mybir.dt.float32
BF16 = mybir.dt.bfloat16
I16 = mybir.dt.int16
AF = mybir.ActivationFunctionType
ALU = mybir.AluOpType

B, D, K = 8192, 1024, 30
M = 8  # cores
BS = B // M  # 1024 rows per core
NSUB = BS // 128  # 8 subtiles of 128 rows
NC_D = D // 128  # 8 d-chunks
KP = 32  # K padded to 32 so gather chunks (128 idx) align with matmul slices
G = 32  # queries per matmul group (one col-tile)
NG = 128 // G  # 4 groups per subtile
GC = G * KP  # 1024 gathered columns per group
NH = 2  # halves per group (PSUM bank limit: 512 f32)
HC = GC // NH  # 512
NIDX = 128 * KP  # 4096 gathered rows per subtile
NCH = NIDX // 128  # 32 gather chunks (128 idx each) per subtile
NCHH = NCH // 2  # 16 chunks per half-subtile gather tile
MASKED_ONES = float(GC - K)  # 994 exp(0)=1 entries per row


def _idx_wrap(vals16):
    """Pack a linear int16 index list into the dma_gather SBUF layout:
    [128, n/16] where linear j sits at (partition j%16, col j//16),
    replicated across the 8 Q7 groups of 16 partitions."""
    n = vals16.shape[0]
    assert n % 16 == 0
    base = vals16.reshape(n // 16, 16).T.astype(np.int16)  # [16, n/16]
    return np.ascontiguousarray(np.tile(base, (8, 1)))  # [128, n/16]


# --------------------------------------------------------------------------
# Launch 1: per-core normalization of the pool shard + pos dots
# --------------------------------------------------------------------------
def build_l1():
    nc = _new_nc()
    zi = nc.dram_tensor("zi", [BS, D], F32, kind="ExternalInput")
    zj = nc.dram_tensor("zj", [BS, D], F32, kind="ExternalInput")
    zin = nc.dram_tensor("zin", [BS, D], BF16, kind="ExternalOutput")
    zjn = nc.dram_tensor("zjn", [BS, D], BF16, kind="ExternalOutput")
    pos8 = nc.dram_tensor("pos8", [128, NSUB], F32, kind="ExternalOutput")

    with tile.TileContext(nc) as tc:
        with (
            tc.tile_pool(name="big", bufs=3) as bigp,
            tc.tile_pool(name="stat", bufs=3) as statp,
            tc.tile_pool(name="posp", bufs=1) as posp,
        ):
            pos_sb = posp.tile([128, NSUB], F32)
            for t in range(NSUB):
                r0 = t * 128
                bi = bigp.tile([128, D], F32, tag="bi")
                nc.sync.dma_start(bi[:, :], zi[r0 : r0 + 128, :])
                bj = bigp.tile([128, D], F32, tag="bj")
                nc.sync.dma_start(bj[:, :], zj[r0 : r0 + 128, :])

                sqdi = bigp.tile([128, D], F32, tag="sqdi")
                ssi = statp.tile([128, 1], F32, tag="ssi")
                nc.scalar.activation(
                    sqdi[:, :], bi[:, :], AF.Square, accum_out=ssi[:, :]
                )
                sqdj = bigp.tile([128, D], F32, tag="sqdj")
                ssj = statp.tile([128, 1], F32, tag="ssj")
                nc.scalar.activation(
                    sqdj[:, :], bj[:, :], AF.Square, accum_out=ssj[:, :]
                )
                nrmi = statp.tile([128, 1], F32, tag="nrmi")
                nc.scalar.activation(nrmi[:, :], ssi[:, :], AF.Sqrt)
                nrmj = statp.tile([128, 1], F32, tag="nrmj")
                nc.scalar.activation(nrmj[:, :], ssj[:, :], AF.Sqrt)
                invi = statp.tile([128, 1], F32, tag="invi")
                nc.vector.reciprocal(invi[:, :], nrmi[:, :])
                invj = statp.tile([128, 1], F32, tag="invj")
                nc.vector.reciprocal(invj[:, :], nrmj[:, :])

                prodd = bigp.tile([128, D], F32, tag="prodd")
                rawdot = statp.tile([128, 1], F32, tag="rawdot")
                nc.vector.tensor_tensor(prodd[:, :], bi[:, :], bj[:, :], ALU.mult)
                nc.vector.tensor_reduce(
                    rawdot[:, :], prodd[:, :], mybir.AxisListType.X, ALU.add
                )

                oi = bigp.tile([128, D], BF16, tag="oi")
                nc.vector.tensor_scalar_mul(oi[:, :], bi[:, :], invi[:, :])
                nc.sync.dma_start(zin[r0 : r0 + 128, :], oi[:, :])
                oj = bigp.tile([128, D], BF16, tag="oj")
                nc.vector.tensor_scalar_mul(oj[:, :], bj[:, :], invj[:, :])
                nc.sync.dma_start(zjn[r0 : r0 + 128, :], oj[:, :])

                tmp = statp.tile([128, 1], F32, tag="tmp")
                nc.vector.tensor_tensor(
                    tmp[:, :], rawdot[:, :], invi[:, :], ALU.mult
                )
                nc.vector.tensor_tensor(
                    pos_sb[:, t : t + 1], tmp[:, :], invj[:, :], ALU.mult
                )
            nc.sync.dma_start(pos8[:, :], pos_sb[:, :])
    nc.compile()
    return nc


# --------------------------------------------------------------------------
# Launch 2: gather + block-diagonal dots + masked softmax-CE
# --------------------------------------------------------------------------
def build_l2():
    nc = _new_nc()
    pool = nc.dram_tensor("pool", [2 * B, D], BF16, kind="ExternalInput")
    idxg_h = nc.dram_tensor("idxg", [128, NSUB * NIDX // 16], I16, kind="ExternalInput")
    idxq_h = nc.dram_tensor("idxq", [128, BS // 16], I16, kind="ExternalInput")
    pos8_h = nc.dram_tensor("pos8", [128, NSUB], F32, kind="ExternalInput")
    mask_h = nc.dram_tensor("mask", [128, GC], F32, kind="ExternalInput")
    trep_h = nc.dram_tensor("trep", [128, 1], F32, kind="ExternalInput")
    out_h = nc.dram_tensor("out", [1, 1], F32, kind="ExternalOutput")

    with tile.TileContext(nc) as tc:
        with (
            tc.tile_pool(name="const", bufs=1) as cp,
            tc.tile_pool(name="gtp", bufs=4) as gtp,
            tc.tile_pool(name="yp", bufs=2) as yp,
            tc.tile_pool(name="dp", bufs=2) as dp,
            tc.tile_pool(name="rowp", bufs=2) as rowp,
            tc.tile_pool(name="psp", bufs=2, space="PSUM") as psp,
            tc.tile_pool(name="psf", bufs=1, space="PSUM") as psfp,
        ):
            idxg = cp.tile([128, NSUB * NIDX // 16], I16)
            nc.sync.dma_start(idxg[:, :], idxg_h[:, :])
            idxq = cp.tile([128, BS // 16], I16)
            nc.sync.dma_start(idxq[:, :], idxq_h[:, :])
            pos8 = cp.tile([128, NSUB], F32)
            nc.sync.dma_start(pos8[:, :], pos8_h[:, :])
            mask = cp.tile([128, GC], F32)
            nc.sync.dma_start(mask[:, :], mask_h[:, :])
            trep = cp.tile([128, 1], F32)
            nc.sync.dma_start(trep[:, :], trep_h[:, :])

            invt = cp.tile([128, 1], F32)
            nc.vector.reciprocal(invt[:, :], trep[:, :])
            ones = cp.tile([128, 1], F32)
            nc.vector.memset(ones[:, :], 1.0)
            s8 = cp.tile([128, NSUB], F32)
            p8 = cp.tile([128, NSUB], F32)

            nc.gpsimd.load_library(mlp)
            nidx_reg = nc.gpsimd.alloc_register("nidx128")
            nc.gpsimd.reg_mov(nidx_reg, 128)
            # queries^T: 8 chunk-gathers of 128 rows; chunk qc==subtile t
            ziT = cp.tile([128, NSUB, NC_D, 128], BF16)
            for qc in range(NSUB):
                nc.gpsimd.dma_gather(
                    ziT[:, qc],
                    pool[:, :],
                    idxq[:, qc * 8 : (qc + 1) * 8],
                    128,
                    nidx_reg,
                    D,
                    transpose=True,
                )

            for t in range(NSUB):
                # two half-subtile gather tiles of 16 chunks x 128 rows each
                gts = []
                for s in range(2):
                    gth = gtp.tile([128, NCHH, NC_D, 128], BF16, tag="gt")
                    for ci in range(NCHH):
                        col0 = (t * NCH + s * NCHH + ci) * 8
                        nc.gpsimd.dma_gather(
                            gth[:, ci],
                            pool[:, :],
                            idxg[:, col0 : col0 + 8],
                            128,
                            nidx_reg,
                            D,
                            transpose=True,
                        )
                    gts.append(gth)
                ps_a = psp.tile([128, HC], F32, tag="psa")
                ps_b = psp.tile([128, HC], F32, tag="psb")
                pss = [ps_a, ps_b]
                for j in range(NG):
                    gth = gts[j // 2]
                    jl = j % 2  # group index within the half tile
                    for h in range(NH):
                        cs = jl * (NCHH // 2) + h * (NCHH // 4)  # 8*jl + 4*h
                        for c in range(NC_D):
                            nc.tensor.matmul(
                                pss[h][j * G : (j + 1) * G, :],
                                ziT[:, t, c, j * G : (j + 1) * G],
                                gth[:, cs : cs + 4, c, :],
                                start=(c == 0),
                                stop=(c == NC_D - 1),
                                tile_position=(0, j * G),
                            )
                # masked exp-sum over both halves
                ss = []
                for h in range(NH):
                    y = yp.tile([128, HC], F32, tag=f"y{h}")
                    nc.vector.tensor_tensor(
                        y[:, :],
                        pss[h][:, :],
                        mask[:, h * HC : (h + 1) * HC],
                        ALU.mult,
                    )
                    ed = dp.tile([128, HC], F32, tag=f"ed{h}")
                    s = rowp.tile([128, 1], F32, tag=f"s{h}")
                    nc.scalar.activation(
                        ed[:, :],
                        y[:, :],
                        AF.Exp,
                        scale=invt[:, :],
                        accum_out=s[:, :],
                    )
                    ss.append(s)
                nc.vector.tensor_scalar_mul(
                    p8[:, t : t + 1], pos8[:, t : t + 1], invt[:, :]
                )
                nc.vector.tensor_tensor(
                    s8[:, t : t + 1], ss[0][:, :], ss[1][:, :], ALU.add
                )

            # row losses, batched: loss = ln(s - 994 + exp(p)) - p
            ep8 = cp.tile([128, NSUB], F32)
            nc.scalar.activation(ep8[:, :], p8[:, :], AF.Exp)
            tot8 = cp.tile([128, NSUB], F32)
            nc.vector.tensor_tensor(tot8[:, :], s8[:, :], ep8[:, :], ALU.add)
            tot28 = cp.tile([128, NSUB], F32)
            nc.vector.tensor_scalar_add(tot28[:, :], tot8[:, :], -MASKED_ONES)
            lg8 = cp.tile([128, NSUB], F32)
            nc.scalar.activation(lg8[:, :], tot28[:, :], AF.Ln)
            d8 = cp.tile([128, NSUB], F32)
            nc.vector.tensor_tensor(d8[:, :], lg8[:, :], p8[:, :], ALU.subtract)
            rowsum = cp.tile([128, 1], F32)
            nc.vector.tensor_reduce(
                rowsum[:, :], d8[:, :], mybir.AxisListType.X, ALU.add
            )

            psf = psfp.tile([128, 16], F32)
            nc.tensor.matmul(
                psf[0:1, 0:1], ones[:, :], rowsum[:, :], start=True, stop=True
            )
            res = cp.tile([1, 1], F32)
            nc.vector.tensor_copy(res[:, :], psf[0:1, 0:1])
            nc.sync.dma_start(out_h[:, :], res[:, :])
    nc.compile()
    return nc


# --------------------------------------------------------------------------
# Host-side driver
# --------------------------------------------------------------------------
def _build_l2_inputs(pool_n, pos8s, neg_indices, temperature):
    """Per-core input dicts for launch 2 (pure data movement)."""
    mask = np.zeros((128, GC), dtype=np.float32)
    p = np.arange(128)[:, None]
    col = np.arange(GC)[None, :]
    mask[((col // KP) == (p % G)) & ((col % KP) < K)] = 1.0
    trep = np.full((128, 1), temperature, dtype=np.float32)

    # subtile pair order: P = b_local*KP + kk, kk >= K padded with kk=0's idx
    X = np.arange(NIDX)
    bl = X // KP  # b_local within subtile
    kk = np.minimum(X % KP, K - 1) * ((X % KP) < K)  # pad slots reuse kk=0

    in_maps = []
    for m in range(M):
        idxg = np.empty((NSUB * NIDX,), dtype=np.int64)
        for t in range(NSUB):
            rows = m * BS + t * 128 + bl
            idxg[t * NIDX : (t + 1) * NIDX] = neg_indices[rows, kk]
        idxg_w = _idx_wrap(idxg.astype(np.int16))
        idxq_w = _idx_wrap((m * BS + np.arange(BS)).astype(np.int16))
        in_maps.append(
            {
                "pool": pool_n,
                "idxg": idxg_w,
                "idxq": idxq_w,
                "pos8": pos8s[m],
                "mask": mask,
                "trep": trep,
            }
        )
    return in_maps


def kernel(z_i, z_j, temperature, neg_indices, _timing=None):
    z_i = np.ascontiguousarray(np.asarray(z_i, dtype=np.float32))
    z_j = np.ascontiguousarray(np.asarray(z_j, dtype=np.float32))
    neg_indices = np.asarray(neg_indices)
    temp = float(np.asarray(temperature))

    cores = list(range(M))
    trace = _timing is not None

    # ---- launch 1
    nc1 = build_l1()
    in1 = [
        {
            "zi": z_i[m * BS : (m + 1) * BS],
            "zj": z_j[m * BS : (m + 1) * BS],
        }
        for m in range(M)
    ]
    r1 = run_bass_kernel_spmd(nc1, in1, cores, trace=trace)
    if trace:
        _timing.append(("l1", r1.exec_time_ns))

    pool_n = np.ascontiguousarray(
        np.concatenate(
            [r1.results[m]["zin"] for m in range(M)]
            + [r1.results[m]["zjn"] for m in range(M)],
            axis=0,
        )
    )
    pos8s = [np.ascontiguousarray(r1.results[m]["pos8"]) for m in range(M)]

    # ---- launch 2
    nc2 = build_l2()
    in2 = _build_l2_inputs(pool_n, pos8s, neg_indices, temp)
    r2 = run_bass_kernel_spmd(nc2, in2, cores, trace=trace)
    if trace:
        _timing.append(("l2", r2.exec_time_ns))

    total = np.float64(0.0)
    for m in range(M):
        total += np.float64(r2.results[m]["out"][0, 0])
    return np.float32(total / B)


# revision 64
# speedup vs baseline: 1.7830x; 1.2959x over previous
"""Trainium2 Bass/Tile kernel for the contrastive (NT-Xent-style) loss.

reference math (B=8192, D=1024, K=30):
    zi = l2norm(z_i); zj = l2norm(z_j)
    pos = rowdot(zi, zj)                      # [B]
    pool = cat(zi, zj)                        # [2B, D]
    neg[b,k] = dot(zi[b], pool[idx[b,k]])     # [B,K]
    logits = cat(pos, neg)/T; loss = mean_b(logsumexp(logits_b) - logits_b0)

Distribution: data-parallel over B across 8 cores (1024 rows each).

Launch 1 (per core): reads its z_i/z_j row shard (f32), computes row norms
(ACT square+fused-rowsum, ACT sqrt, DVE reciprocal), writes bf16 normalized
shards + f32 pos-dots [128, 8] (partition=row%128, col=row//128). The host
concatenates the 16 shard outputs into the full normalized pool (pure data
movement, no math).

Launch 2 (per core): the full normalized bf16 pool is replicated; for each
128-row subtile one dma_gather(transpose=True) pulls its 3840 negative rows
in d-transposed layout [128=d%128, 8=d//128, 3840]. TensorE computes
block-diagonal query/negative dots: 4 groups of 32 queries vs their 960
gathered columns, output col-tiled to PSUM partitions 32j..32j+31, contracted
over 8 d-chunks, split in two 480-wide halves (PSUM bank limit). The PSUM is
multiplied by a 0/1 mask (zeroing the off-diagonal waste entries), then
ScalarE computes exp(x/T) with a fused row-sum. Off-diagonal zeros contribute
exp(0)=1 each, so row_loss = log(exp(p/T) + sum - 930) - p/T. Row losses
accumulate across subtiles and reduce to one scalar per core via a
ones-vector matmul; the host sums 8 scalars and divides by B.
"""

import sys

for _p in ("/opt/trn_rl_repo",):
    if _p not in sys.path:
        sys.path.insert(0, _p)

import numpy as np
import ml_dtypes  # noqa: F401  (bf16 numpy dtype registration)

import concourse.bacc as bacc
import concourse.bass as bass  # noqa: F401
import concourse.mybir as mybir
import concourse.tile as tile
from concourse.bass_utils import run_bass_kernel_spmd
from concourse.library_config import mlp


def _new_nc():
    return bacc.Bacc("TRN2", target_bir_lowering=False, debug=False)

F32 = mybir.dt.float32
BF16 = mybir.dt.bfloat16
I16 = mybir.dt.int16
AF = mybir.ActivationFunctionType
ALU = mybir.AluOpType

B, D, K = 8192, 1024, 30
M = 8  # cores
BS = B // M  # 1024 rows per core
NSUB = BS // 128  # 8 subtiles of 128 rows
NC_D = D // 128  # 8 d-chunks
KK_SPLITS = [(0, 8), (8, 16), (16, 24), (24, 30)]  # HW gather limit: 1024 idx
ACT_KK = 20  # negatives whose row-sum reduction runs on ScalarE (rest on DVE)


def _idx_wrap(vals16):
    """Pack a linear int16 index list into the dma_gather SBUF layout:
    [128, n/16] where linear j sits at (partition j%16, col j//16),
    replicated across the 8 Q7 groups of 16 partitions."""
    n = vals16.shape[0]
    assert n % 16 == 0
    base = vals16.reshape(n // 16, 16).T.astype(np.int16)  # [16, n/16]
    return np.ascontiguousarray(np.tile(base, (8, 1)))  # [128, n/16]


# --------------------------------------------------------------------------
# Launch 1: per-core normalization of the pool shard + pos dots
# --------------------------------------------------------------------------
def build_l1():
    nc = _new_nc()
    zi = nc.dram_tensor("zi", [BS, D], F32, kind="ExternalInput")
    zj = nc.dram_tensor("zj", [BS, D], F32, kind="ExternalInput")
    zin = nc.dram_tensor("zin", [BS, D], BF16, kind="ExternalOutput")
    zjn = nc.dram_tensor("zjn", [BS, D], BF16, kind="ExternalOutput")
    pos8 = nc.dram_tensor("pos8", [128, NSUB], F32, kind="ExternalOutput")

    with tile.TileContext(nc) as tc:
        with (
            tc.tile_pool(name="big", bufs=3) as bigp,
            tc.tile_pool(name="stat", bufs=3) as statp,
            tc.tile_pool(name="posp", bufs=1) as posp,
        ):
            pos_sb = posp.tile([128, NSUB], F32)
            for t in range(NSUB):
                r0 = t * 128
                bi = bigp.tile([128, D], F32, tag="bi")
                nc.sync.dma_start(bi[:, :], zi[r0 : r0 + 128, :])
                bj = bigp.tile([128, D], F32, tag="bj")
                nc.sync.dma_start(bj[:, :], zj[r0 : r0 + 128, :])

                sqdi = bigp.tile([128, D], F32, tag="sqdi")
                ssi = statp.tile([128, 1], F32, tag="ssi")
                nc.scalar.activation(
                    sqdi[:, :], bi[:, :], AF.Square, accum_out=ssi[:, :]
                )
                sqdj = bigp.tile([128, D], F32, tag="sqdj")
                ssj = statp.tile([128, 1], F32, tag="ssj")
                nc.scalar.activation(
                    sqdj[:, :], bj[:, :], AF.Square, accum_out=ssj[:, :]
                )
                nrmi = statp.tile([128, 1], F32, tag="nrmi")
                nc.scalar.activation(nrmi[:, :], ssi[:, :], AF.Sqrt)
                nrmj = statp.tile([128, 1], F32, tag="nrmj")
                nc.scalar.activation(nrmj[:, :], ssj[:, :], AF.Sqrt)
                invi = statp.tile([128, 1], F32, tag="invi")
                nc.vector.reciprocal(invi[:, :], nrmi[:, :])
                invj = statp.tile([128, 1], F32, tag="invj")
                nc.vector.reciprocal(invj[:, :], nrmj[:, :])

                prodd = bigp.tile([128, D], F32, tag="prodd")
                rawdot = statp.tile([128, 1], F32, tag="rawdot")
                nc.vector.tensor_tensor(prodd[:, :], bi[:, :], bj[:, :], ALU.mult)
                nc.vector.tensor_reduce(
                    rawdot[:, :], prodd[:, :], mybir.AxisListType.X, ALU.add
                )

                oi = bigp.tile([128, D], BF16, tag="oi")
                nc.vector.tensor_scalar_mul(oi[:, :], bi[:, :], invi[:, :])
                nc.sync.dma_start(zin[r0 : r0 + 128, :], oi[:, :])
                oj = bigp.tile([128, D], BF16, tag="oj")
                nc.vector.tensor_scalar_mul(oj[:, :], bj[:, :], invj[:, :])
                nc.sync.dma_start(zjn[r0 : r0 + 128, :], oj[:, :])

                tmp = statp.tile([128, 1], F32, tag="tmp")
                nc.vector.tensor_tensor(
                    tmp[:, :], rawdot[:, :], invi[:, :], ALU.mult
                )
                nc.vector.tensor_tensor(
                    pos_sb[:, t : t + 1], tmp[:, :], invj[:, :], ALU.mult
                )
            nc.sync.dma_start(pos8[:, :], pos_sb[:, :])
    nc.compile()
    return nc


# --------------------------------------------------------------------------
# Launch 2: gather + block-diagonal dots + masked softmax-CE
# --------------------------------------------------------------------------
def build_l2():
    nc = _new_nc()
    pool = nc.dram_tensor("pool", [2 * B, D], BF16, kind="ExternalInput")
    ziq_h = nc.dram_tensor("ziq", [BS, D], BF16, kind="ExternalInput")
    idxg_h = nc.dram_tensor(
        "idxg", [128, NSUB * 128 * K // 16], I16, kind="ExternalInput"
    )
    pos8_h = nc.dram_tensor("pos8", [128, NSUB], F32, kind="ExternalInput")
    trep_h = nc.dram_tensor("trep", [128, 1], F32, kind="ExternalInput")
    out_h = nc.dram_tensor("out", [1, 1], F32, kind="ExternalOutput")

    with tile.TileContext(nc) as tc:
        with (
            tc.tile_pool(name="const", bufs=1) as cp,
            tc.tile_pool(name="gtp", bufs=2) as gtp,
            tc.tile_pool(name="qp", bufs=2) as qp,
            tc.tile_pool(name="pp", bufs=3) as ppool,
            tc.tile_pool(name="dp", bufs=3) as dp,
            tc.tile_pool(name="rowp", bufs=2) as rowp,
            tc.tile_pool(name="psf", bufs=1, space="PSUM") as psfp,
        ):
            idxg = cp.tile([128, NSUB * 128 * K // 16], I16)
            nc.sync.dma_start(idxg[:, :], idxg_h[:, :])
            pos8 = cp.tile([128, NSUB], F32)
            nc.sync.dma_start(pos8[:, :], pos8_h[:, :])
            trep = cp.tile([128, 1], F32)
            nc.sync.dma_start(trep[:, :], trep_h[:, :])

            invt = cp.tile([128, 1], F32)
            nc.vector.reciprocal(invt[:, :], trep[:, :])
            ones = cp.tile([128, 1], F32)
            nc.vector.memset(ones[:, :], 1.0)
            s8 = cp.tile([128, NSUB], F32)
            p8 = cp.tile([128, NSUB], F32)

            nc.gpsimd.load_library(mlp)
            nreg = {}
            for _, (k0, k1) in enumerate(KK_SPLITS):
                n = 128 * (k1 - k0)
                if n not in nreg:
                    r = nc.gpsimd.alloc_register(f"nidx{n}")
                    nc.gpsimd.reg_mov(r, n)
                    nreg[n] = r

            for t in range(NSUB):
                # negatives in rows layout: G[b, kk, :] = pool[idx[row_b, kk]]
                gt = gtp.tile([128, K, D], BF16, tag="gt")
                col = t * (128 * K // 16)
                for k0, k1 in KK_SPLITS:
                    n = 128 * (k1 - k0)
                    nc.gpsimd.dma_gather(
                        gt[:, k0:k1, :],
                        pool[:, :],
                        idxg[:, col : col + n // 16],
                        n,
                        nreg[n],
                        D,
                        transpose=False,
                    )
                    col += n // 16
                q = qp.tile([128, D], BF16, tag="q")
                nc.sync.dma_start(q[:, :], ziq_h[t * 128 : (t + 1) * 128, :])
                negsub = rowp.tile([128, K], F32, tag="negsub")
                for kk in range(K):
                    pc = ppool.tile([128, D], BF16, tag="pc")
                    nc.vector.tensor_tensor(
                        pc[:, :], gt[:, kk, :], q[:, :], ALU.mult
                    )
                    if kk < ACT_KK:
                        edm = dp.tile([128, D], BF16, tag="edm")
                        nc.scalar.activation(
                            edm[:, :],
                            pc[:, :],
                            AF.Copy,
                            accum_out=negsub[:, kk : kk + 1],
                        )
                    else:
                        edm = dp.tile([128, D], BF16, tag="edm2")
                        nc.vector.tensor_scalar(
                            edm[:, :],
                            pc[:, :],
                            1.0,
                            None,
                            ALU.mult,
                            ALU.add,
                            accum_out=negsub[:, kk : kk + 1],
                        )
                # exp-sum of the 30 negatives (fused row-sum on ACT)
                e30 = rowp.tile([128, K], F32, tag="e30")
                nc.scalar.activation(
                    e30[:, :],
                    negsub[:, :],
                    AF.Exp,
                    scale=invt[:, :],
                    accum_out=s8[:, t : t + 1],
                )
                nc.vector.tensor_scalar_mul(
                    p8[:, t : t + 1], pos8[:, t : t + 1], invt[:, :]
                )

            # row losses, batched: loss = ln(s + exp(p)) - p
            ep8 = cp.tile([128, NSUB], F32)
            nc.scalar.activation(ep8[:, :], p8[:, :], AF.Exp)
            tot8 = cp.tile([128, NSUB], F32)
            nc.vector.tensor_tensor(tot8[:, :], s8[:, :], ep8[:, :], ALU.add)
            lg8 = cp.tile([128, NSUB], F32)
            nc.scalar.activation(lg8[:, :], tot8[:, :], AF.Ln)
            d8 = cp.tile([128, NSUB], F32)
            nc.vector.tensor_tensor(d8[:, :], lg8[:, :], p8[:, :], ALU.subtract)
            rowsum = cp.tile([128, 1], F32)
            nc.vector.tensor_reduce(
                rowsum[:, :], d8[:, :], mybir.AxisListType.X, ALU.add
            )

            psf = psfp.tile([128, 16], F32)
            nc.tensor.matmul(
                psf[0:1, 0:1], ones[:, :], rowsum[:, :], start=True, stop=True
            )
            res = cp.tile([1, 1], F32)
            nc.vector.tensor_copy(res[:, :], psf[0:1, 0:1])
            nc.sync.dma_start(out_h[:, :], res[:, :])
    nc.compile()
    return nc


# --------------------------------------------------------------------------
# Host-side driver
# --------------------------------------------------------------------------
def _build_l2_inputs(pool_n, pos8s, neg_indices, temperature):
    """Per-core input dicts for launch 2 (pure data movement)."""
    trep = np.full((128, 1), temperature, dtype=np.float32)

    in_maps = []
    for m in range(M):
        idxg = np.empty((NSUB * 128 * K,), dtype=np.int64)
        pos = 0
        for t in range(NSUB):
            for k0, k1 in KK_SPLITS:
                n = 128 * (k1 - k0)
                i = np.arange(n)
                rows = m * BS + t * 128 + (i % 128)
                idxg[pos : pos + n] = neg_indices[rows, k0 + i // 128]
                pos += n
        idxg_w = _idx_wrap(idxg.astype(np.int16))
        in_maps.append(
            {
                "pool": pool_n,
                "ziq": np.ascontiguousarray(pool_n[m * BS : (m + 1) * BS]),
                "idxg": idxg_w,
                "pos8": pos8s[m],
                "trep": trep,
            }
        )
    return in_maps


def kernel(z_i, z_j, temperature, neg_indices, _timing=None):
    z_i = np.ascontiguousarray(np.asarray(z_i, dtype=np.float32))
    z_j = np.ascontiguousarray(np.asarray(z_j, dtype=np.float32))
    neg_indices = np.asarray(neg_indices)
    temp = float(np.asarray(temperature))

    cores = list(range(M))
    trace = _timing is not None

    # ---- launch 1
    nc1 = build_l1()
    in1 = [
        {
            "zi": z_i[m * BS : (m + 1) * BS],
            "zj": z_j[m * BS : (m + 1) * BS],
        }
        for m in range(M)
    ]
    r1 = run_bass_kernel_spmd(nc1, in1, cores, trace=trace)
    if trace:
        _timing.append(("l1", r1.exec_time_ns))

    pool_n = np.ascontiguousarray(
        np.concatenate(
            [r1.results[m]["zin"] for m in range(M)]
            + [r1.results[m]["zjn"] for m in range(M)],
            axis=0,
        )
    )
    pos8s = [np.ascontiguousarray(r1.results[m]["pos8"]) for m in range(M)]

    # ---- launch 2
    nc2 = build_l2()
    in2 = _build_l2_inputs(pool_n, pos8s, neg_indices, temp)
    r2 = run_bass_kernel_spmd(nc2, in2, cores, trace=trace)
    if trace:
        _timing.append(("l2", r2.exec_time_ns))

    total = np.float64(0.0)
    for m in range(M):
        total += np.float64(r2.results[m]["out"][0, 0])
    return np.float32(total / B)


# revision 65
# speedup vs baseline: 1.8621x; 1.0444x over previous
"""Trainium2 Bass/Tile kernel for the contrastive (NT-Xent-style) loss.

reference math (B=8192, D=1024, K=30):
    zi = l2norm(z_i); zj = l2norm(z_j)
    pos = rowdot(zi, zj)                      # [B]
    pool = cat(zi, zj)                        # [2B, D]
    neg[b,k] = dot(zi[b], pool[idx[b,k]])     # [B,K]
    logits = cat(pos, neg)/T; loss = mean_b(logsumexp(logits_b) - logits_b0)

Distribution: data-parallel over B across 8 cores (1024 rows each).

Launch 1 (per core): reads its z_i/z_j row shard (f32), computes row norms
(ACT square+fused-rowsum, ACT sqrt, DVE reciprocal), writes bf16 normalized
shards + f32 pos-dots [128, 8] (partition=row%128, col=row//128). The host
concatenates the 16 shard outputs into the full normalized pool (pure data
movement, no math).

Launch 2 (per core): the full normalized bf16 pool is replicated; for each
128-row subtile one dma_gather(transpose=True) pulls its 3840 negative rows
in d-transposed layout [128=d%128, 8=d//128, 3840]. TensorE computes
block-diagonal query/negative dots: 4 groups of 32 queries vs their 960
gathered columns, output col-tiled to PSUM partitions 32j..32j+31, contracted
over 8 d-chunks, split in two 480-wide halves (PSUM bank limit). The PSUM is
multiplied by a 0/1 mask (zeroing the off-diagonal waste entries), then
ScalarE computes exp(x/T) with a fused row-sum. Off-diagonal zeros contribute
exp(0)=1 each, so row_loss = log(exp(p/T) + sum - 930) - p/T. Row losses
accumulate across subtiles and reduce to one scalar per core via a
ones-vector matmul; the host sums 8 scalars and divides by B.
"""

import sys

for _p in ("/opt/trn_rl_repo",):
    if _p not in sys.path:
        sys.path.insert(0, _p)

import numpy as np
import ml_dtypes  # noqa: F401  (bf16 numpy dtype registration)

import concourse.bacc as bacc
import concourse.bass as bass  # noqa: F401
import concourse.mybir as mybir
import concourse.tile as tile
from concourse.bass_utils import run_bass_kernel_spmd
from concourse.library_config import mlp


def _new_nc():
    return bacc.Bacc("TRN2", target_bir_lowering=False, debug=False)

F32 = mybir.dt.float32
BF16 = mybir.dt.bfloat16
I16 = mybir.dt.int16
AF = mybir.ActivationFunctionType
ALU = mybir.AluOpType

B, D, K = 8192, 1024, 30
M = 8  # cores
BS = B // M  # 1024 rows per core
NSUB = BS // 128  # 8 subtiles of 128 rows
NC_D = D // 128  # 8 d-chunks
KK_SPLITS = [(0, 8), (8, 16), (16, 24), (24, 30)]  # HW gather limit: 1024 idx
ACT_KK = 24  # negatives whose row-sum reduction runs on ScalarE (rest on DVE)


def _idx_wrap(vals16):
    """Pack a linear int16 index list into the dma_gather SBUF layout:
    [128, n/16] where linear j sits at (partition j%16, col j//16),
    replicated across the 8 Q7 groups of 16 partitions."""
    n = vals16.shape[0]
    assert n % 16 == 0
    base = vals16.reshape(n // 16, 16).T.astype(np.int16)  # [16, n/16]
    return np.ascontiguousarray(np.tile(base, (8, 1)))  # [128, n/16]


# --------------------------------------------------------------------------
# Launch 1: per-core normalization of the pool shard + pos dots
# --------------------------------------------------------------------------
def build_l1():
    nc = _new_nc()
    zi = nc.dram_tensor("zi", [BS, D], F32, kind="ExternalInput")
    zj = nc.dram_tensor("zj", [BS, D], F32, kind="ExternalInput")
    zin = nc.dram_tensor("zin", [BS, D], BF16, kind="ExternalOutput")
    zjn = nc.dram_tensor("zjn", [BS, D], BF16, kind="ExternalOutput")
    pos8 = nc.dram_tensor("pos8", [128, NSUB], F32, kind="ExternalOutput")

    with tile.TileContext(nc) as tc:
        with (
            tc.tile_pool(name="big", bufs=3) as bigp,
            tc.tile_pool(name="stat", bufs=3) as statp,
            tc.tile_pool(name="posp", bufs=1) as posp,
        ):
            pos_sb = posp.tile([128, NSUB], F32)
            for t in range(NSUB):
                r0 = t * 128
                bi = bigp.tile([128, D], F32, tag="bi")
                nc.sync.dma_start(bi[:, :], zi[r0 : r0 + 128, :])
                bj = bigp.tile([128, D], F32, tag="bj")
                nc.sync.dma_start(bj[:, :], zj[r0 : r0 + 128, :])

                sqdi = bigp.tile([128, D], F32, tag="sqdi")
                ssi = statp.tile([128, 1], F32, tag="ssi")
                nc.scalar.activation(
                    sqdi[:, :], bi[:, :], AF.Square, accum_out=ssi[:, :]
                )
                sqdj = bigp.tile([128, D], F32, tag="sqdj")
                ssj = statp.tile([128, 1], F32, tag="ssj")
                nc.scalar.activation(
                    sqdj[:, :], bj[:, :], AF.Square, accum_out=ssj[:, :]
                )
                nrmi = statp.tile([128, 1], F32, tag="nrmi")
                nc.scalar.activation(nrmi[:, :], ssi[:, :], AF.Sqrt)
                nrmj = statp.tile([128, 1], F32, tag="nrmj")
                nc.scalar.activation(nrmj[:, :], ssj[:, :], AF.Sqrt)
                invi = statp.tile([128, 1], F32, tag="invi")
                nc.vector.reciprocal(invi[:, :], nrmi[:, :])
                invj = statp.tile([128, 1], F32, tag="invj")
                nc.vector.reciprocal(invj[:, :], nrmj[:, :])

                prodd = bigp.tile([128, D], F32, tag="prodd")
                rawdot = statp.tile([128, 1], F32, tag="rawdot")
                nc.vector.tensor_tensor(prodd[:, :], bi[:, :], bj[:, :], ALU.mult)
                nc.vector.tensor_reduce(
                    rawdot[:, :], prodd[:, :], mybir.AxisListType.X, ALU.add
                )

                oi = bigp.tile([128, D], BF16, tag="oi")
                nc.vector.tensor_scalar_mul(oi[:, :], bi[:, :], invi[:, :])
                nc.sync.dma_start(zin[r0 : r0 + 128, :], oi[:, :])
                oj = bigp.tile([128, D], BF16, tag="oj")
                nc.vector.tensor_scalar_mul(oj[:, :], bj[:, :], invj[:, :])
                nc.sync.dma_start(zjn[r0 : r0 + 128, :], oj[:, :])

                tmp = statp.tile([128, 1], F32, tag="tmp")
                nc.vector.tensor_tensor(
                    tmp[:, :], rawdot[:, :], invi[:, :], ALU.mult
                )
                nc.vector.tensor_tensor(
                    pos_sb[:, t : t + 1], tmp[:, :], invj[:, :], ALU.mult
                )
            nc.sync.dma_start(pos8[:, :], pos_sb[:, :])
    nc.compile()
    return nc


# --------------------------------------------------------------------------
# Launch 2: gather + block-diagonal dots + masked softmax-CE
# --------------------------------------------------------------------------
def build_l2():
    nc = _new_nc()
    pool = nc.dram_tensor("pool", [2 * B, D], BF16, kind="ExternalInput")
    ziq_h = nc.dram_tensor("ziq", [BS, D], BF16, kind="ExternalInput")
    idxg_h = nc.dram_tensor(
        "idxg", [128, NSUB * 128 * K // 16], I16, kind="ExternalInput"
    )
    pos8_h = nc.dram_tensor("pos8", [128, NSUB], F32, kind="ExternalInput")
    trep_h = nc.dram_tensor("trep", [128, 1], F32, kind="ExternalInput")
    out_h = nc.dram_tensor("out", [1, 1], F32, kind="ExternalOutput")

    with tile.TileContext(nc) as tc:
        with (
            tc.tile_pool(name="const", bufs=1) as cp,
            tc.tile_pool(name="gtp", bufs=2) as gtp,
            tc.tile_pool(name="qp", bufs=2) as qp,
            tc.tile_pool(name="pp", bufs=3) as ppool,
            tc.tile_pool(name="dp", bufs=3) as dp,
            tc.tile_pool(name="rowp", bufs=2) as rowp,
            tc.tile_pool(name="psf", bufs=1, space="PSUM") as psfp,
        ):
            idxg = cp.tile([128, NSUB * 128 * K // 16], I16)
            nc.sync.dma_start(idxg[:, :], idxg_h[:, :])
            pos8 = cp.tile([128, NSUB], F32)
            nc.sync.dma_start(pos8[:, :], pos8_h[:, :])
            trep = cp.tile([128, 1], F32)
            nc.sync.dma_start(trep[:, :], trep_h[:, :])

            invt = cp.tile([128, 1], F32)
            nc.vector.reciprocal(invt[:, :], trep[:, :])
            ones = cp.tile([128, 1], F32)
            nc.vector.memset(ones[:, :], 1.0)
            s8 = cp.tile([128, NSUB], F32)
            p8 = cp.tile([128, NSUB], F32)

            nc.gpsimd.load_library(mlp)
            nreg = {}
            for _, (k0, k1) in enumerate(KK_SPLITS):
                n = 128 * (k1 - k0)
                if n not in nreg:
                    r = nc.gpsimd.alloc_register(f"nidx{n}")
                    nc.gpsimd.reg_mov(r, n)
                    nreg[n] = r

            for t in range(NSUB):
                # negatives in rows layout: G[b, kk, :] = pool[idx[row_b, kk]]
                gt = gtp.tile([128, K, D], BF16, tag="gt")
                col = t * (128 * K // 16)
                for k0, k1 in KK_SPLITS:
                    n = 128 * (k1 - k0)
                    nc.gpsimd.dma_gather(
                        gt[:, k0:k1, :],
                        pool[:, :],
                        idxg[:, col : col + n // 16],
                        n,
                        nreg[n],
                        D,
                        transpose=False,
                    )
                    col += n // 16
                q = qp.tile([128, D], BF16, tag="q")
                nc.sync.dma_start(q[:, :], ziq_h[t * 128 : (t + 1) * 128, :])
                negsub = rowp.tile([128, K], F32, tag="negsub")
                for kk in range(K):
                    pc = ppool.tile([128, D], BF16, tag="pc")
                    nc.vector.tensor_tensor(
                        pc[:, :], gt[:, kk, :], q[:, :], ALU.mult
                    )
                    if kk < ACT_KK:
                        edm = dp.tile([128, D], BF16, tag="edm")
                        nc.scalar.activation(
                            edm[:, :],
                            pc[:, :],
                            AF.Copy,
                            accum_out=negsub[:, kk : kk + 1],
                        )
                    else:
                        edm = dp.tile([128, D], BF16, tag="edm2")
                        nc.vector.tensor_scalar(
                            edm[:, :],
                            pc[:, :],
                            1.0,
                            None,
                            ALU.mult,
                            ALU.add,
                            accum_out=negsub[:, kk : kk + 1],
                        )
                # exp-sum of the 30 negatives (fused row-sum on ACT)
                e30 = rowp.tile([128, K], F32, tag="e30")
                nc.scalar.activation(
                    e30[:, :],
                    negsub[:, :],
                    AF.Exp,
                    scale=invt[:, :],
                    accum_out=s8[:, t : t + 1],
                )
                nc.vector.tensor_scalar_mul(
                    p8[:, t : t + 1], pos8[:, t : t + 1], invt[:, :]
                )

            # row losses, batched: loss = ln(s + exp(p)) - p
            ep8 = cp.tile([128, NSUB], F32)
            nc.scalar.activation(ep8[:, :], p8[:, :], AF.Exp)
            tot8 = cp.tile([128, NSUB], F32)
            nc.vector.tensor_tensor(tot8[:, :], s8[:, :], ep8[:, :], ALU.add)
            lg8 = cp.tile([128, NSUB], F32)
            nc.scalar.activation(lg8[:, :], tot8[:, :], AF.Ln)
            d8 = cp.tile([128, NSUB], F32)
            nc.vector.tensor_tensor(d8[:, :], lg8[:, :], p8[:, :], ALU.subtract)
            rowsum = cp.tile([128, 1], F32)
            nc.vector.tensor_reduce(
                rowsum[:, :], d8[:, :], mybir.AxisListType.X, ALU.add
            )

            psf = psfp.tile([128, 16], F32)
            nc.tensor.matmul(
                psf[0:1, 0:1], ones[:, :], rowsum[:, :], start=True, stop=True
            )
            res = cp.tile([1, 1], F32)
            nc.vector.tensor_copy(res[:, :], psf[0:1, 0:1])
            nc.sync.dma_start(out_h[:, :], res[:, :])
    nc.compile()
    return nc


# --------------------------------------------------------------------------
# Host-side driver
# --------------------------------------------------------------------------
def _build_l2_inputs(pool_n, pos8s, neg_indices, temperature):
    """Per-core input dicts for launch 2 (pure data movement)."""
    trep = np.full((128, 1), temperature, dtype=np.float32)

    in_maps = []
    for m in range(M):
        idxg = np.empty((NSUB * 128 * K,), dtype=np.int64)
        pos = 0
        for t in range(NSUB):
            for k0, k1 in KK_SPLITS:
                n = 128 * (k1 - k0)
                i = np.arange(n)
                rows = m * BS + t * 128 + (i % 128)
                idxg[pos : pos + n] = neg_indices[rows, k0 + i // 128]
                pos += n
        idxg_w = _idx_wrap(idxg.astype(np.int16))
        in_maps.append(
            {
                "pool": pool_n,
                "ziq": np.ascontiguousarray(pool_n[m * BS : (m + 1) * BS]),
                "idxg": idxg_w,
                "pos8": pos8s[m],
                "trep": trep,
            }
        )
    return in_maps


def kernel(z_i, z_j, temperature, neg_indices, _timing=None):
    z_i = np.ascontiguousarray(np.asarray(z_i, dtype=np.float32))
    z_j = np.ascontiguousarray(np.asarray(z_j, dtype=np.float32))
    neg_indices = np.asarray(neg_indices)
    temp = float(np.asarray(temperature))

    cores = list(range(M))
    trace = _timing is not None

    # ---- launch 1
    nc1 = build_l1()
    in1 = [
        {
            "zi": z_i[m * BS : (m + 1) * BS],
            "zj": z_j[m * BS : (m + 1) * BS],
        }
        for m in range(M)
    ]
    r1 = run_bass_kernel_spmd(nc1, in1, cores, trace=trace)
    if trace:
        _timing.append(("l1", r1.exec_time_ns))

    pool_n = np.ascontiguousarray(
        np.concatenate(
            [r1.results[m]["zin"] for m in range(M)]
            + [r1.results[m]["zjn"] for m in range(M)],
            axis=0,
        )
    )
    pos8s = [np.ascontiguousarray(r1.results[m]["pos8"]) for m in range(M)]

    # ---- launch 2
    nc2 = build_l2()
    in2 = _build_l2_inputs(pool_n, pos8s, neg_indices, temp)
    r2 = run_bass_kernel_spmd(nc2, in2, cores, trace=trace)
    if trace:
        _timing.append(("l2", r2.exec_time_ns))

    total = np.float64(0.0)
    for m in range(M):
        total += np.float64(r2.results[m]["out"][0, 0])
    return np.float32(total / B)
